# revision 11
# baseline (speedup 1.0000x reference)
"""Trainium2 Bass kernel for nn_Attention_Critic (gnn_message_passing).

Strategy (8-way batch data parallel, 4096 samples/core):
  - Host fuses weights: WeQ=We@Wq, WeK=We@Wk, WeV=We@Wv (encoder folded into
    the Q/K/V projections; enc is materialized only for agent 0),
    WoW1b=Wo@W1[256:], b1f=b1+bo@W1[256:] (fc_out folded into l1).
  - Final LN+l3 folded algebraically: res = rstd*(W3.qr - mean*sum(W3)) + b3,
    with mean/var of qr from ones/W3 matvecs (float32r).
  - Per 512-sample chunk: LN via bn_stats + tensor_scalar (sample-major),
    PE-transpose to feature-major [feat, sample] bf16, projections as
    128x128-tiled bf16 matmuls, attention scores via elementwise QK + a
    block-ones matmul that reduces over head dims AND broadcasts the score
    back across them, exp on ScalarE, k=7 reductions as bf16 add-trees.
"""

import contextlib

import numpy as np
import ml_dtypes

import concourse.bass as bass
import concourse.tile as tile
from concourse import bacc, mybir
from concourse.bass_utils import run_bass_kernel_spmd
from concourse.masks import make_identity

AF = mybir.ActivationFunctionType
OP = mybir.AluOpType
BF = mybir.dt.bfloat16
F32 = mybir.dt.float32
F32R = mybir.dt.float32r
USE_LRELU = True   # HW ACT table has Lrelu; CoreSim does not (test_sim flips)

B, A, S, D, H, NH, HD = 32768, 8, 256, 256, 256, 4, 64
EPS = 1e-5
NCORES = 8
BC = B // NCORES          # 4096 samples per core
NB = 512                  # samples per chunk
NCH = BC // NB            # 8 chunks per core
NW = 1794                 # fused bf16 weight columns
NBI = 7                   # f32 bias/vec slots

# wcat column offsets (each 256 wide)
C_WE, C_WQ, C_WK, C_WV, C_W1A, C_WO1B, C_L, C_W3O = (
    0, 256, 512, 768, 1024, 1280, 1536, 1792)
# bcat slots: biases 0-4, W3 at 5, ones at 6
B_BE, B_BQ, B_BK, B_BV, B_B1F, B_W3, B_ONE = 0, 1, 2, 3, 4, 5, 6


def build_kernel(tc, nch=NCH):
    nc = tc.nc
    s_in = nc.dram_tensor("s", [nch * NB, A * S], F32, kind="ExternalInput").ap()
    wcat = nc.dram_tensor("wcat", [128, 2, NW], BF, kind="ExternalInput").ap()
    bcat = nc.dram_tensor("bcat", [128, 2, NBI], F32, kind="ExternalInput").ap()
    scal = nc.dram_tensor("scal", [1, 2], F32, kind="ExternalInput").ap()
    out = nc.dram_tensor("out", [nch * NB], F32, kind="ExternalOutput").ap()

    with contextlib.ExitStack() as ctx:
        const = ctx.enter_context(tc.tile_pool(name="const", bufs=1))
        spool = ctx.enter_context(tc.tile_pool(name="spool", bufs=5))
        apool = ctx.enter_context(tc.tile_pool(name="apool", bufs=8))
        tpool = ctx.enter_context(tc.tile_pool(name="tpool", bufs=2))
        mmout = ctx.enter_context(tc.tile_pool(name="mmout", bufs=2))
        kvpool = ctx.enter_context(tc.tile_pool(name="kvpool", bufs=1))
        qkpool = ctx.enter_context(tc.tile_pool(name="qkpool", bufs=2))
        trpool = ctx.enter_context(tc.tile_pool(name="trpool", bufs=2))
        fpool = ctx.enter_context(tc.tile_pool(name="fpool", bufs=1))
        psA = ctx.enter_context(tc.tile_pool(name="psA", bufs=2, space="PSUM"))
        psT = ctx.enter_context(tc.tile_pool(name="psT", bufs=2, space="PSUM"))
        psF = ctx.enter_context(tc.tile_pool(name="psF", bufs=1, space="PSUM"))

        wtile = const.tile([128, 2, NW], BF)
        nc.sync.dma_start(wtile[:], wcat)
        btile = const.tile([128, 2, NBI], F32)
        nc.sync.dma_start(btile[:], bcat)
        eps_t = const.tile([128, 1], F32)
        nc.vector.memset(eps_t[:], EPS)
        ident = const.tile([128, 128], BF)
        make_identity(nc, ident[:])
        sw_t = const.tile([128, 1], F32)   # sum(W3) broadcast over partitions
        nc.gpsimd.dma_start(sw_t[:], scal[0:1, 0:1].to_broadcast((128, 1)))
        b3_t = const.tile([128, 1], F32)   # b3 broadcast
        nc.gpsimd.dma_start(b3_t[:], scal[0:1, 1:2].to_broadcast((128, 1)))
        # per-chunk stat rows: [p, {W3.qr, sum qr, sum qr^2}, chunk, j]
        Fp = const.tile([128, 3, nch, 4], F32)

        def W(col, ks, mc=0, width=128):
            return wtile[:, ks, col + mc * 128: col + mc * 128 + width]

        for c in range(nch):
            # ---- Stage A: load (f32->bf16 cast in DMA) + LayerNorm ----
            sN = []
            for bt in range(4):
                st = spool.tile([128, A * S], BF, tag="s_in")
                nc.gpsimd.dma_start(
                    st[:], s_in[c * NB + bt * 128: c * NB + (bt + 1) * 128, :])
                stats = apool.tile([128, 4, 6], F32, tag="stats")
                for g in range(4):
                    nc.vector.bn_stats(stats[:, g, :], st[:, g * 512:(g + 1) * 512])
                mv = apool.tile([128, 2], F32, tag="mv")
                nc.vector.bn_aggr(mv[:], stats[:])
                rt = apool.tile([128, 1], F32, tag="rt")
                nc.scalar.activation(rt[:], mv[:, 1:2], AF.Sqrt, bias=eps_t[:])
                nc.vector.reciprocal(rt[:], rt[:])
                sn = spool.tile([128, A * S], BF, tag="sn")
                nc.gpsimd.tensor_scalar(
                    sn[:], st[:], scalar1=mv[:, 0:1], scalar2=rt[:],
                    op0=OP.subtract, op1=OP.mult)
                sN.append(sn)

            # ---- Stage T: PE transpose to feature-major ----
            # snT[p, fb, bt, bb] = sn_bt[bb, fb*128+p]; feature f = fb*128+p,
            # fb = 2*a + ks (a=agent, ks=K-half); sample index = bt*128+bb
            snT = tpool.tile([128, 16, 4, 128], BF, tag="snT")
            for fb in range(16):
                pt = psT.tile([128, 4, 128], BF, tag="ptrans")
                for bt in range(4):
                    nc.tensor.transpose(
                        pt[:, bt, :], sN[bt][:, fb * 128:(fb + 1) * 128], ident[:])
                if fb % 2 == 0:
                    nc.scalar.activation(snT[:, fb], pt[:], AF.Copy)
                else:
                    nc.vector.tensor_copy(snT[:, fb], pt[:])

            def rhs_s(ks, a):
                return snT[:, 2 * a + ks]   # [128, 4, 128] -> N=512

            # ---- projections (all contract over the 256 encoder inputs) ----
            def proj256(colbase, a, bias_slot, dst_mc_ap, func=AF.Identity):
                for mc in range(2):
                    ps = psA.tile([128, 2, NB], F32, tag="psmm")
                    for ks in range(2):
                        nc.tensor.matmul(
                            ps[:, 0], W(colbase, ks, mc), rhs_s(ks, a),
                            start=(ks == 0), stop=(ks == 1))
                    nc.scalar.activation(
                        dst_mc_ap(mc), ps[:, 0], func,
                        bias=btile[:, mc, bias_slot:bias_slot + 1])

            s_iT = mmout.tile([128, 2, NB], BF, tag="s_iT")
            proj256(C_WE, 0, B_BE, lambda mc: s_iT[:, mc])
            qT = mmout.tile([128, 2, NB], BF, tag="qT")
            proj256(C_WQ, 0, B_BQ, lambda mc: qT[:, mc])

            kT = kvpool.tile([128, 2, 7, NB], BF, tag="kT")
            vT = kvpool.tile([128, 2, 7, NB], BF, tag="vT")
            # a-pairs share one 2-bank PSUM tile and a single wide eviction
            apairs = [(1, 2), (3, 4), (5, 6), (7,)]
            for ap_ in apairs:
                for mc in range(2):
                    na = len(ap_)
                    psk = psA.tile([128, 2, NB], F32, tag="psmm")
                    for j, a in enumerate(ap_):
                        for ks in range(2):
                            nc.tensor.matmul(
                                psk[:, j], W(C_WK, ks, mc), rhs_s(ks, a),
                                start=(ks == 0), stop=(ks == 1))
                    nc.scalar.activation(
                        kT[:, mc, ap_[0] - 1:ap_[0] - 1 + na], psk[:, :na],
                        AF.Identity, bias=btile[:, mc, B_BK:B_BK + 1])
                    psv = psA.tile([128, 2, NB], F32, tag="psmm")
                    for j, a in enumerate(ap_):
                        for ks in range(2):
                            nc.tensor.matmul(
                                psv[:, j], W(C_WV, ks, mc), rhs_s(ks, a),
                                start=(ks == 0), stop=(ks == 1))
                    if USE_LRELU:
                        nc.scalar.activation(
                            vT[:, mc, ap_[0] - 1:ap_[0] - 1 + na], psv[:, :na],
                            AF.Lrelu, bias=btile[:, mc, B_BV:B_BV + 1],
                            alpha=0.01)
                    else:
                        # leaky_relu(t, .01) = max(t, .01*t), t = x + bv
                        dst = vT[:, mc, ap_[0] - 1:ap_[0] - 1 + na]
                        nc.scalar.activation(
                            dst, psv[:, :na], AF.Identity,
                            bias=btile[:, mc, B_BV:B_BV + 1])
                        t2 = trpool.tile([128, 2, NB], BF, tag="vt2")
                        nc.vector.tensor_scalar_mul(t2[:, :na], dst, 0.01)
                        nc.vector.tensor_max(dst, dst, t2[:, :na])

            # ---- attention ----
            qk = qkpool.tile([128, 2, 7, NB], BF, tag="qku")
            for mc in range(2):
                for k in range(7):
                    nc.vector.tensor_mul(qk[:, mc, k], kT[:, mc, k], qT[:, mc])
            # block-ones matmul: reduces QK over each head's 64 dims and
            # broadcasts the score back across them (diag chunks only)
            eb = kvpool.tile([128, 2, 7, NB], BF, tag="eb")
            kpairs = [(0, 1), (2, 3), (4, 5), (6,)]
            for mc in range(2):
                for kp in kpairs:
                    nk = len(kp)
                    pss = psA.tile([128, 2, NB], F32, tag="psmm")
                    for j, k in enumerate(kp):
                        nc.tensor.matmul(
                            pss[:, j], W(C_L, mc, mc), qk[:, mc, k],
                            start=True, stop=True)
                    nc.scalar.activation(
                        eb[:, mc, kp[0]:kp[0] + nk], pss[:, :nk], AF.Exp,
                        scale=1.0 / np.sqrt(HD))

            u = qkpool.tile([128, 2, 7, NB], BF, tag="qku")
            for mc in range(2):
                for k in range(7):
                    nc.vector.tensor_mul(u[:, mc, k], eb[:, mc, k], vT[:, mc, k])

            def ktree(src, dst):
                for mc in range(2):
                    t0 = trpool.tile([128, NB], BF, tag="tr0")
                    nc.gpsimd.tensor_add(t0[:], src[:, mc, 0], src[:, mc, 1])
                    t1 = trpool.tile([128, NB], BF, tag="tr1")
                    nc.gpsimd.tensor_add(t1[:], src[:, mc, 2], src[:, mc, 3])
                    t2 = trpool.tile([128, NB], BF, tag="tr2")
                    nc.gpsimd.tensor_add(t2[:], src[:, mc, 4], src[:, mc, 5])
                    nc.gpsimd.tensor_add(t0[:], t0[:], t1[:])
                    nc.gpsimd.tensor_add(t2[:], t2[:], src[:, mc, 6])
                    nc.gpsimd.tensor_add(dst[:, mc], t0[:], t2[:])

            avU = mmout.tile([128, 2, NB], BF, tag="avU")
            ktree(u, avU)
            sumB = mmout.tile([128, 2, NB], BF, tag="sumB")
            ktree(eb, sumB)
            rs = mmout.tile([128, 2, NB], BF, tag="rs")
            with nc.allow_low_precision(reason="softmax denom, bf16 ok at 2e-2"):
                nc.vector.reciprocal(rs[:], sumB[:])
            avT = mmout.tile([128, 2, NB], BF, tag="avT")
            for mc in range(2):
                nc.vector.tensor_mul(avT[:, mc], avU[:, mc], rs[:, mc])

            # ---- l1 fused with fc_out: qr = relu(W1a.s_i + WoW1b.av + b1f) ----
            qr = mmout.tile([128, 2, NB], BF, tag="qr")
            for mc in range(2):
                ps = psA.tile([128, 2, NB], F32, tag="psmm")
                for ks in range(2):
                    nc.tensor.matmul(ps[:, 0], W(C_W1A, ks, mc), s_iT[:, ks],
                                     start=(ks == 0), stop=False)
                for ks in range(2):
                    nc.tensor.matmul(ps[:, 0], W(C_WO1B, ks, mc), avT[:, ks],
                                     start=False, stop=(ks == 1))
                nc.scalar.activation(qr[:, mc], ps[:, 0], AF.Relu,
                                     bias=btile[:, mc, B_B1F:B_B1F + 1])
            qr2 = mmout.tile([128, 2, NB], BF, tag="qr2")
            nc.gpsimd.tensor_mul(qr2[:], qr[:], qr[:])

            # ---- final LN+l3 stats via float32r matvecs ----
            ps1 = psF.tile([2, NB], F32, tag="ps1")
            for ks in range(2):
                nc.tensor.matmul(
                    ps1[:], W(C_W3O, ks, 0, width=2), qr[:, ks],
                    start=(ks == 0), stop=(ks == 1))
            ps2 = psF.tile([1, NB], F32, tag="ps2")
            for ks in range(2):
                nc.tensor.matmul(
                    ps2[:], W(C_W3O + 1, ks, 0, width=1), qr2[:, ks],
                    start=(ks == 0), stop=(ks == 1))
            stmp1 = fpool.tile([2, NB], F32, tag="stmp1")
            nc.scalar.activation(stmp1[:], ps1[:], AF.Copy)
            stmp2 = fpool.tile([1, NB], F32, tag="stmp2")
            nc.scalar.activation(stmp2[:], ps2[:], AF.Copy)
            # scatter row [1, 512] -> Fp[:, r, c, :] (sample = p*4 + j; the DMA
            # pairs the flat source stream with the partition-major dest)
            nc.gpsimd.dma_start(Fp[:, 0, c, :], stmp1[0:1, :])
            nc.gpsimd.dma_start(Fp[:, 1, c, :], stmp1[1:2, :])
            nc.gpsimd.dma_start(Fp[:, 2, c, :], stmp2[0:1, :])

        # ---- final LN+l3 math on [128, nch*4] ----
        FW = nch * 4
        w3qr = Fp[:, 0].rearrange("p c j -> p (c j)")
        sq = Fp[:, 1].rearrange("p c j -> p (c j)")
        sq2 = Fp[:, 2].rearrange("p c j -> p (c j)")
        m = fpool.tile([128, FW], F32, tag="fm")
        nc.scalar.mul(m[:], sq, 1.0 / H)
        ex2 = fpool.tile([128, FW], F32, tag="fe")
        nc.scalar.mul(ex2[:], sq2, 1.0 / H)
        var = fpool.tile([128, FW], F32, tag="fv")
        nc.vector.tensor_mul(var[:], m[:], m[:])
        nc.vector.tensor_sub(var[:], ex2[:], var[:])
        rstd = fpool.tile([128, FW], F32, tag="fr")
        nc.scalar.activation(rstd[:], var[:], AF.Sqrt, bias=eps_t[:])
        nc.vector.reciprocal(rstd[:], rstd[:])
        msw = fpool.tile([128, FW], F32, tag="fw")
        nc.vector.tensor_scalar_mul(msw[:], m[:], sw_t[:])
        res = fpool.tile([128, FW], F32, tag="fres")
        nc.vector.tensor_sub(res[:], w3qr, msw[:])
        nc.vector.tensor_mul(res[:], res[:], rstd[:])
        nc.vector.tensor_scalar_add(res[:], res[:], b3_t[:])
        nc.sync.dma_start(
            out.rearrange("(c p j) -> p c j", p=128, j=4),
            res.rearrange("p (c j) -> p c j", j=4))
    return nc


def _prepare_host(We, be, Wq, Wk, Wv, bv, Wo, bo, W1, b1, W3, b3):
    f = lambda x: np.asarray(x, dtype=np.float32)
    We, be, Wq, Wk, Wv, bv = f(We), f(be), f(Wq), f(Wk), f(Wv), f(bv)
    Wo, bo, W1, b1, W3, b3 = f(Wo), f(bo), f(W1), f(b1), f(W3), f(b3)
    WeQ, beQ = We @ Wq, be @ Wq
    WeK, beK = We @ Wk, be @ Wk
    WeV, beV = We @ Wv, be @ Wv + bv
    W1a, W1b = W1[:D], W1[D:]
    WoW1b, b1f = Wo @ W1b, b1 + bo @ W1b
    L = np.zeros((H, H), np.float32)
    for n in range(NH):
        L[n * HD:(n + 1) * HD, n * HD:(n + 1) * HD] = 1.0
    w3o = np.zeros((H, 2), np.float32)
    w3o[:, 0] = W3[:, 0]
    w3o[:, 1] = 1.0
    wfull = np.concatenate([We, WeQ, WeK, WeV, W1a, WoW1b, L, w3o], axis=1)
    assert wfull.shape == (256, NW)
    wcat = np.ascontiguousarray(
        wfull.reshape(2, 128, NW).transpose(1, 0, 2)).astype(ml_dtypes.bfloat16)
    ones = np.ones(H, np.float32)
    bfull = np.stack([be, beQ, beK, beV, b1f, W3[:, 0], ones], axis=1)
    assert bfull.shape == (256, NBI)
    bcat = np.ascontiguousarray(bfull.reshape(2, 128, NBI).transpose(1, 0, 2))
    scal = np.array([[W3.sum(), b3[0]]], np.float32)
    return wcat, bcat, scal


_CACHED = {}


def _get_compiled(nch=NCH, num_devices=1):
    key = (nch, num_devices)
    if key not in _CACHED:
        nc = bacc.Bacc("TRN2", target_bir_lowering=False, debug=False,
                       num_devices=num_devices)
        with tile.TileContext(nc) as tc:
            build_kernel(tc, nch=nch)
        nc.compile()
        _CACHED[key] = nc
    return _CACHED[key]


def kernel(s, We, be, Wq, Wk, Wv, bv, Wo, bo, W1, b1, W3, b3, _trace=False):
    s = np.asarray(s, dtype=np.float32)
    wcat, bcat, scal = _prepare_host(We, be, Wq, Wk, Wv, bv, Wo, bo, W1, b1,
                                     W3, b3)
    nc = _get_compiled()
    in_maps = []
    for i in range(NCORES):
        shard = np.ascontiguousarray(s[i * BC:(i + 1) * BC])
        in_maps.append({"s": shard, "wcat": wcat, "bcat": bcat, "scal": scal})
    res = run_bass_kernel_spmd(nc, in_maps, core_ids=list(range(NCORES)),
                               trace=_trace)
    outs = [np.asarray(r["out"], np.float32).reshape(BC, 1)
            for r in res.results]
    full = np.concatenate(outs, axis=0)
    if _trace:
        return full, res
    return full


# revision 12
# speedup vs baseline: 2.7592x; 2.7592x over previous
"""Trainium2 Bass kernel for nn_Attention_Critic (gnn_message_passing).

Strategy (8-way batch data parallel, 4096 samples/core):
  - Host fuses weights: WeQ=We@Wq, WeK=We@Wk, WeV=We@Wv (encoder folded into
    the Q/K/V projections; enc is materialized only for agent 0),
    WoW1b=Wo@W1[256:], b1f=b1+bo@W1[256:] (fc_out folded into l1).
  - Final LN+l3 folded algebraically: res = rstd*(W3.qr - mean*sum(W3)) + b3,
    with mean/var of qr from ones/W3 matvecs (float32r).
  - Per 512-sample chunk: LN via bn_stats + tensor_scalar (sample-major),
    PE-transpose to feature-major [feat, sample] bf16, projections as
    128x128-tiled bf16 matmuls, attention scores via elementwise QK + a
    block-ones matmul that reduces over head dims AND broadcasts the score
    back across them, exp on ScalarE, k=7 reductions as bf16 add-trees.
"""

import contextlib

import numpy as np
import ml_dtypes

import concourse.bass as bass
import concourse.tile as tile
from concourse import bacc, mybir
from concourse.bass_utils import run_bass_kernel_spmd
from concourse.masks import make_identity

AF = mybir.ActivationFunctionType
OP = mybir.AluOpType
BF = mybir.dt.bfloat16
F32 = mybir.dt.float32
F32R = mybir.dt.float32r
USE_LRELU = True   # HW ACT table has Lrelu; CoreSim does not (test_sim flips)

B, A, S, D, H, NH, HD = 32768, 8, 256, 256, 256, 4, 64
EPS = 1e-5
NCORES = 8
BC = B // NCORES          # 4096 samples per core
NB = 512                  # samples per chunk
NCH = BC // NB            # 8 chunks per core
NW = 1794                 # fused bf16 weight columns
NBI = 7                   # f32 bias/vec slots

# wcat column offsets (each 256 wide)
C_WE, C_WQ, C_WK, C_WV, C_W1A, C_WO1B, C_L, C_W3O = (
    0, 256, 512, 768, 1024, 1280, 1536, 1792)
# bcat slots: biases 0-4, W3 at 5, ones at 6
B_BE, B_BQ, B_BK, B_BV, B_B1F, B_W3, B_ONE = 0, 1, 2, 3, 4, 5, 6


def build_kernel(tc, nch=NCH):
    nc = tc.nc
    s_in = nc.dram_tensor("s", [nch * NB, A * S], F32, kind="ExternalInput").ap()
    wcat = nc.dram_tensor("wcat", [128, 2, NW], BF, kind="ExternalInput").ap()
    bcat = nc.dram_tensor("bcat", [128, 2, NBI], F32, kind="ExternalInput").ap()
    scal = nc.dram_tensor("scal", [1, 2], F32, kind="ExternalInput").ap()
    out = nc.dram_tensor("out", [nch * NB], F32, kind="ExternalOutput").ap()

    with contextlib.ExitStack() as ctx:
        const = ctx.enter_context(tc.tile_pool(name="const", bufs=1))
        spool = ctx.enter_context(tc.tile_pool(name="spool", bufs=5))
        apool = ctx.enter_context(tc.tile_pool(name="apool", bufs=8))
        tpool = ctx.enter_context(tc.tile_pool(name="tpool", bufs=2))
        mmout = ctx.enter_context(tc.tile_pool(name="mmout", bufs=2))
        kvpool = ctx.enter_context(tc.tile_pool(name="kvpool", bufs=1))
        qkpool = ctx.enter_context(tc.tile_pool(name="qkpool", bufs=2))
        trpool = ctx.enter_context(tc.tile_pool(name="trpool", bufs=2))
        fpool = ctx.enter_context(tc.tile_pool(name="fpool", bufs=1))
        psA = ctx.enter_context(tc.tile_pool(name="psA", bufs=2, space="PSUM"))
        psT = ctx.enter_context(tc.tile_pool(name="psT", bufs=2, space="PSUM"))
        psF = ctx.enter_context(tc.tile_pool(name="psF", bufs=1, space="PSUM"))

        wtile = const.tile([128, 2, NW], BF)
        nc.sync.dma_start(wtile[:], wcat)
        btile = const.tile([128, 2, NBI], F32)
        nc.sync.dma_start(btile[:], bcat)
        eps_t = const.tile([128, 1], F32)
        nc.vector.memset(eps_t[:], EPS)
        ident = const.tile([128, 128], BF)
        make_identity(nc, ident[:])
        sw_t = const.tile([128, 1], F32)   # sum(W3) broadcast over partitions
        nc.gpsimd.dma_start(sw_t[:], scal[0:1, 0:1].to_broadcast((128, 1)))
        b3_t = const.tile([128, 1], F32)   # b3 broadcast
        nc.gpsimd.dma_start(b3_t[:], scal[0:1, 1:2].to_broadcast((128, 1)))
        # per-chunk stat rows: [p, {W3.qr, sum qr, sum qr^2}, chunk, j]
        Fp = const.tile([128, 3, nch, 4], F32)

        def W(col, ks, mc=0, width=128):
            return wtile[:, ks, col + mc * 128: col + mc * 128 + width]

        for c in range(nch):
            # ---- Stage A: load (f32->bf16 cast in DMA) + LayerNorm ----
            sN = []
            for bt in range(4):
                st = spool.tile([128, A * S], BF, tag="s_in")
                nc.gpsimd.dma_start(
                    st[:], s_in[c * NB + bt * 128: c * NB + (bt + 1) * 128, :])
                stats = apool.tile([128, 4, 6], F32, tag="stats")
                for g in range(4):
                    nc.vector.bn_stats(stats[:, g, :], st[:, g * 512:(g + 1) * 512])
                mv = apool.tile([128, 2], F32, tag="mv")
                nc.vector.bn_aggr(mv[:], stats[:])
                rt = apool.tile([128, 1], F32, tag="rt")
                nc.scalar.activation(rt[:], mv[:, 1:2], AF.Ln, bias=eps_t[:])
                nc.scalar.activation(rt[:], rt[:], AF.Exp, scale=-0.5)
                sn = spool.tile([128, A * S], BF, tag="sn")
                nc.vector.tensor_scalar(
                    sn[:], st[:], scalar1=mv[:, 0:1], scalar2=rt[:],
                    op0=OP.subtract, op1=OP.mult)
                sN.append(sn)

            # ---- Stage T: PE transpose to feature-major ----
            # snT[p, fb, bt, bb] = sn_bt[bb, fb*128+p]; feature f = fb*128+p,
            # fb = 2*a + ks (a=agent, ks=K-half); sample index = bt*128+bb
            snT = tpool.tile([128, 16, 4, 128], BF, tag="snT")
            for fb in range(16):
                pt = psT.tile([128, 4, 128], BF, tag="ptrans")
                for bt in range(4):
                    nc.tensor.transpose(
                        pt[:, bt, :], sN[bt][:, fb * 128:(fb + 1) * 128], ident[:])
                if fb % 2 == 0:
                    nc.scalar.activation(snT[:, fb], pt[:], AF.Copy)
                else:
                    nc.vector.tensor_copy(snT[:, fb], pt[:])

            def rhs_s(ks, a):
                return snT[:, 2 * a + ks]   # [128, 4, 128] -> N=512

            # ---- projections (all contract over the 256 encoder inputs) ----
            def proj256(colbase, a, bias_slot, dst_mc_ap, func=AF.Identity):
                for mc in range(2):
                    ps = psA.tile([128, 2, NB], F32, tag="psmm")
                    for ks in range(2):
                        nc.tensor.matmul(
                            ps[:, 0], W(colbase, ks, mc), rhs_s(ks, a),
                            start=(ks == 0), stop=(ks == 1))
                    nc.scalar.activation(
                        dst_mc_ap(mc), ps[:, 0], func,
                        bias=btile[:, mc, bias_slot:bias_slot + 1])

            s_iT = mmout.tile([128, 2, NB], BF, tag="s_iT")
            proj256(C_WE, 0, B_BE, lambda mc: s_iT[:, mc])
            qT = mmout.tile([128, 2, NB], BF, tag="qT")
            proj256(C_WQ, 0, B_BQ, lambda mc: qT[:, mc])

            kT = kvpool.tile([128, 2, 7, NB], BF, tag="kT")
            vT = kvpool.tile([128, 2, 7, NB], BF, tag="vT")
            # a-pairs share one 2-bank PSUM tile and a single wide eviction
            apairs = [(1, 2), (3, 4), (5, 6), (7,)]
            for ap_ in apairs:
                for mc in range(2):
                    na = len(ap_)
                    psk = psA.tile([128, 2, NB], F32, tag="psmm")
                    for j, a in enumerate(ap_):
                        for ks in range(2):
                            nc.tensor.matmul(
                                psk[:, j], W(C_WK, ks, mc), rhs_s(ks, a),
                                start=(ks == 0), stop=(ks == 1))
                    nc.scalar.activation(
                        kT[:, mc, ap_[0] - 1:ap_[0] - 1 + na], psk[:, :na],
                        AF.Identity, bias=btile[:, mc, B_BK:B_BK + 1])
                    psv = psA.tile([128, 2, NB], F32, tag="psmm")
                    for j, a in enumerate(ap_):
                        for ks in range(2):
                            nc.tensor.matmul(
                                psv[:, j], W(C_WV, ks, mc), rhs_s(ks, a),
                                start=(ks == 0), stop=(ks == 1))
                    if USE_LRELU:
                        nc.scalar.activation(
                            vT[:, mc, ap_[0] - 1:ap_[0] - 1 + na], psv[:, :na],
                            AF.Lrelu, bias=btile[:, mc, B_BV:B_BV + 1],
                            alpha=0.01)
                    else:
                        # leaky_relu(t, .01) = max(t, .01*t), t = x + bv
                        dst = vT[:, mc, ap_[0] - 1:ap_[0] - 1 + na]
                        nc.scalar.activation(
                            dst, psv[:, :na], AF.Identity,
                            bias=btile[:, mc, B_BV:B_BV + 1])
                        t2 = trpool.tile([128, 2, NB], BF, tag="vt2")
                        nc.vector.tensor_scalar_mul(t2[:, :na], dst, 0.01)
                        nc.vector.tensor_max(dst, dst, t2[:, :na])

            # ---- attention ----
            qk = qkpool.tile([128, 2, 7, NB], BF, tag="qku")
            for k in range(7):
                nc.vector.tensor_mul(qk[:, :, k], kT[:, :, k], qT[:])
            # block-ones matmul: reduces QK over each head's 64 dims and
            # broadcasts the score back across them (diag chunks only)
            eb = kvpool.tile([128, 2, 7, NB], BF, tag="eb")
            kpairs = [(0, 1), (2, 3), (4, 5), (6,)]
            for mc in range(2):
                for kp in kpairs:
                    nk = len(kp)
                    pss = psA.tile([128, 2, NB], F32, tag="psmm")
                    for j, k in enumerate(kp):
                        nc.tensor.matmul(
                            pss[:, j], W(C_L, mc, mc), qk[:, mc, k],
                            start=True, stop=True)
                    nc.scalar.activation(
                        eb[:, mc, kp[0]:kp[0] + nk], pss[:, :nk], AF.Exp,
                        scale=1.0 / np.sqrt(HD))

            u = qkpool.tile([128, 2, 7, NB], BF, tag="qku")
            for k in range(7):
                nc.vector.tensor_mul(u[:, :, k], eb[:, :, k], vT[:, :, k])

            def ktree(src, dst):
                t0 = trpool.tile([128, 2, NB], BF, tag="tr0")
                nc.vector.tensor_add(t0[:], src[:, :, 0], src[:, :, 1])
                t1 = trpool.tile([128, 2, NB], BF, tag="tr1")
                nc.vector.tensor_add(t1[:], src[:, :, 2], src[:, :, 3])
                t2 = trpool.tile([128, 2, NB], BF, tag="tr2")
                nc.vector.tensor_add(t2[:], src[:, :, 4], src[:, :, 5])
                nc.vector.tensor_add(t0[:], t0[:], t1[:])
                nc.vector.tensor_add(t2[:], t2[:], src[:, :, 6])
                nc.vector.tensor_add(dst[:], t0[:], t2[:])

            avU = mmout.tile([128, 2, NB], BF, tag="avU")
            ktree(u, avU)
            sumB = mmout.tile([128, 2, NB], BF, tag="sumB")
            ktree(eb, sumB)
            rs = mmout.tile([128, 2, NB], BF, tag="rs")
            nc.scalar.activation(rs[:], sumB[:], AF.Ln)
            nc.scalar.activation(rs[:], rs[:], AF.Exp, scale=-1.0)
            avT = mmout.tile([128, 2, NB], BF, tag="avT")
            nc.vector.tensor_mul(avT[:], avU[:], rs[:])

            # ---- l1 fused with fc_out: qr = relu(W1a.s_i + WoW1b.av + b1f) ----
            qr = mmout.tile([128, 2, NB], BF, tag="qr")
            for mc in range(2):
                ps = psA.tile([128, 2, NB], F32, tag="psmm")
                for ks in range(2):
                    nc.tensor.matmul(ps[:, 0], W(C_W1A, ks, mc), s_iT[:, ks],
                                     start=(ks == 0), stop=False)
                for ks in range(2):
                    nc.tensor.matmul(ps[:, 0], W(C_WO1B, ks, mc), avT[:, ks],
                                     start=False, stop=(ks == 1))
                nc.scalar.activation(qr[:, mc], ps[:, 0], AF.Relu,
                                     bias=btile[:, mc, B_B1F:B_B1F + 1])
            qr2 = mmout.tile([128, 2, NB], BF, tag="qr2")
            nc.scalar.activation(qr2[:], qr[:], AF.Square)

            # ---- final LN+l3 stats via float32r matvecs ----
            ps1 = psF.tile([2, NB], F32, tag="ps1")
            for ks in range(2):
                nc.tensor.matmul(
                    ps1[:], W(C_W3O, ks, 0, width=2), qr[:, ks],
                    start=(ks == 0), stop=(ks == 1))
            ps2 = psF.tile([1, NB], F32, tag="ps2")
            for ks in range(2):
                nc.tensor.matmul(
                    ps2[:], W(C_W3O + 1, ks, 0, width=1), qr2[:, ks],
                    start=(ks == 0), stop=(ks == 1))
            stmp1 = fpool.tile([2, NB], F32, tag="stmp1")
            nc.scalar.activation(stmp1[:], ps1[:], AF.Copy)
            stmp2 = fpool.tile([1, NB], F32, tag="stmp2")
            nc.scalar.activation(stmp2[:], ps2[:], AF.Copy)
            # scatter row [1, 512] -> Fp[:, r, c, :] (sample = p*4 + j; the DMA
            # pairs the flat source stream with the partition-major dest)
            nc.gpsimd.dma_start(Fp[:, 0, c, :], stmp1[0:1, :])
            nc.gpsimd.dma_start(Fp[:, 1, c, :], stmp1[1:2, :])
            nc.gpsimd.dma_start(Fp[:, 2, c, :], stmp2[0:1, :])

        # ---- final LN+l3 math on [128, nch*4] ----
        FW = nch * 4
        w3qr = Fp[:, 0].rearrange("p c j -> p (c j)")
        sq = Fp[:, 1].rearrange("p c j -> p (c j)")
        sq2 = Fp[:, 2].rearrange("p c j -> p (c j)")
        m = fpool.tile([128, FW], F32, tag="fm")
        nc.scalar.mul(m[:], sq, 1.0 / H)
        ex2 = fpool.tile([128, FW], F32, tag="fe")
        nc.scalar.mul(ex2[:], sq2, 1.0 / H)
        var = fpool.tile([128, FW], F32, tag="fv")
        nc.vector.tensor_mul(var[:], m[:], m[:])
        nc.vector.tensor_sub(var[:], ex2[:], var[:])
        rstd = fpool.tile([128, FW], F32, tag="fr")
        nc.scalar.activation(rstd[:], var[:], AF.Ln, bias=eps_t[:])
        nc.scalar.activation(rstd[:], rstd[:], AF.Exp, scale=-0.5)
        msw = fpool.tile([128, FW], F32, tag="fw")
        nc.vector.tensor_scalar_mul(msw[:], m[:], sw_t[:])
        res = fpool.tile([128, FW], F32, tag="fres")
        nc.vector.tensor_sub(res[:], w3qr, msw[:])
        nc.vector.tensor_mul(res[:], res[:], rstd[:])
        nc.vector.tensor_scalar_add(res[:], res[:], b3_t[:])
        nc.sync.dma_start(
            out.rearrange("(c p j) -> p c j", p=128, j=4),
            res.rearrange("p (c j) -> p c j", j=4))
    return nc


def _prepare_host(We, be, Wq, Wk, Wv, bv, Wo, bo, W1, b1, W3, b3):
    f = lambda x: np.asarray(x, dtype=np.float32)
    We, be, Wq, Wk, Wv, bv = f(We), f(be), f(Wq), f(Wk), f(Wv), f(bv)
    Wo, bo, W1, b1, W3, b3 = f(Wo), f(bo), f(W1), f(b1), f(W3), f(b3)
    WeQ, beQ = We @ Wq, be @ Wq
    WeK, beK = We @ Wk, be @ Wk
    WeV, beV = We @ Wv, be @ Wv + bv
    W1a, W1b = W1[:D], W1[D:]
    WoW1b, b1f = Wo @ W1b, b1 + bo @ W1b
    L = np.zeros((H, H), np.float32)
    for n in range(NH):
        L[n * HD:(n + 1) * HD, n * HD:(n + 1) * HD] = 1.0
    w3o = np.zeros((H, 2), np.float32)
    w3o[:, 0] = W3[:, 0]
    w3o[:, 1] = 1.0
    wfull = np.concatenate([We, WeQ, WeK, WeV, W1a, WoW1b, L, w3o], axis=1)
    assert wfull.shape == (256, NW)
    wcat = np.ascontiguousarray(
        wfull.reshape(2, 128, NW).transpose(1, 0, 2)).astype(ml_dtypes.bfloat16)
    ones = np.ones(H, np.float32)
    bfull = np.stack([be, beQ, beK, beV, b1f, W3[:, 0], ones], axis=1)
    assert bfull.shape == (256, NBI)
    bcat = np.ascontiguousarray(bfull.reshape(2, 128, NBI).transpose(1, 0, 2))
    scal = np.array([[W3.sum(), b3[0]]], np.float32)
    return wcat, bcat, scal


_CACHED = {}


def _get_compiled(nch=NCH, num_devices=1):
    key = (nch, num_devices)
    if key not in _CACHED:
        nc = bacc.Bacc("TRN2", target_bir_lowering=False, debug=False,
                       num_devices=num_devices)
        with tile.TileContext(nc) as tc:
            build_kernel(tc, nch=nch)
        nc.compile()
        _CACHED[key] = nc
    return _CACHED[key]


def kernel(s, We, be, Wq, Wk, Wv, bv, Wo, bo, W1, b1, W3, b3, _trace=False):
    s = np.asarray(s, dtype=np.float32)
    wcat, bcat, scal = _prepare_host(We, be, Wq, Wk, Wv, bv, Wo, bo, W1, b1,
                                     W3, b3)
    nc = _get_compiled()
    in_maps = []
    for i in range(NCORES):
        shard = np.ascontiguousarray(s[i * BC:(i + 1) * BC])
        in_maps.append({"s": shard, "wcat": wcat, "bcat": bcat, "scal": scal})
    res = run_bass_kernel_spmd(nc, in_maps, core_ids=list(range(NCORES)),
                               trace=_trace)
    outs = [np.asarray(r["out"], np.float32).reshape(BC, 1)
            for r in res.results]
    full = np.concatenate(outs, axis=0)
    if _trace:
        return full, res
    return full


# revision 13
# speedup vs baseline: 3.2442x; 1.1758x over previous
"""Trainium2 Bass kernel for nn_Attention_Critic (gnn_message_passing).

Strategy (8-way batch data parallel, 4096 samples/core):
  - Host fuses weights: WeQ=We@Wq, WeK=We@Wk, WeV=We@Wv (encoder folded into
    the Q/K/V projections; enc is materialized only for agent 0),
    WoW1b=Wo@W1[256:], b1f=b1+bo@W1[256:] (fc_out folded into l1).
  - Final LN+l3 folded algebraically: res = rstd*(W3.qr - mean*sum(W3)) + b3,
    with mean/var of qr from ones/W3 matvecs (float32r).
  - Per 512-sample chunk: LN via bn_stats + tensor_scalar (sample-major),
    PE-transpose to feature-major [feat, sample] bf16, projections as
    128x128-tiled bf16 matmuls, attention scores via elementwise QK + a
    block-ones matmul that reduces over head dims AND broadcasts the score
    back across them, exp on ScalarE, k=7 reductions as bf16 add-trees.
"""

import contextlib

import numpy as np
import ml_dtypes

import concourse.bass as bass
import concourse.tile as tile
from concourse import bacc, mybir
from concourse.bass_utils import run_bass_kernel_spmd
from concourse.masks import make_identity

AF = mybir.ActivationFunctionType
OP = mybir.AluOpType
BF = mybir.dt.bfloat16
F32 = mybir.dt.float32
F32R = mybir.dt.float32r
USE_LRELU = False  # Lrelu lives in its own ACT table set; set-switches cost 1.3us each

B, A, S, D, H, NH, HD = 32768, 8, 256, 256, 256, 4, 64
EPS = 1e-5
NCORES = 8
BC = B // NCORES          # 4096 samples per core
NB = 512                  # samples per chunk
NCH = BC // NB            # 8 chunks per core
NW = 1794                 # fused bf16 weight columns
NBI = 7                   # f32 bias/vec slots

# wcat column offsets (each 256 wide)
C_WE, C_WQ, C_WK, C_WV, C_W1A, C_WO1B, C_L, C_W3O = (
    0, 256, 512, 768, 1024, 1280, 1536, 1792)
# bcat slots: biases 0-4, W3 at 5, ones at 6
B_BE, B_BQ, B_BK, B_BV, B_B1F, B_W3, B_ONE = 0, 1, 2, 3, 4, 5, 6


def build_kernel(tc, nch=NCH):
    nc = tc.nc
    s_in = nc.dram_tensor("s", [nch * NB, A * S], F32, kind="ExternalInput").ap()
    wcat = nc.dram_tensor("wcat", [128, 2, NW], BF, kind="ExternalInput").ap()
    bcat = nc.dram_tensor("bcat", [128, 2, NBI], F32, kind="ExternalInput").ap()
    scal = nc.dram_tensor("scal", [1, 2], F32, kind="ExternalInput").ap()
    out = nc.dram_tensor("out", [nch * NB], F32, kind="ExternalOutput").ap()

    with contextlib.ExitStack() as ctx:
        const = ctx.enter_context(tc.tile_pool(name="const", bufs=1))
        spool = ctx.enter_context(tc.tile_pool(name="spool", bufs=5))
        apool = ctx.enter_context(tc.tile_pool(name="apool", bufs=8))
        tpool = ctx.enter_context(tc.tile_pool(name="tpool", bufs=2))
        mmout = ctx.enter_context(tc.tile_pool(name="mmout", bufs=2))
        kvpool = ctx.enter_context(tc.tile_pool(name="kvpool", bufs=1))
        qkpool = ctx.enter_context(tc.tile_pool(name="qkpool", bufs=2))
        trpool = ctx.enter_context(tc.tile_pool(name="trpool", bufs=2))
        fpool = ctx.enter_context(tc.tile_pool(name="fpool", bufs=1))
        psA = ctx.enter_context(tc.tile_pool(name="psA", bufs=2, space="PSUM"))
        psT = ctx.enter_context(tc.tile_pool(name="psT", bufs=2, space="PSUM"))
        psF = ctx.enter_context(tc.tile_pool(name="psF", bufs=1, space="PSUM"))

        wtile = const.tile([128, 2, NW], BF)
        nc.sync.dma_start(wtile[:], wcat)
        btile = const.tile([128, 2, NBI], F32)
        nc.sync.dma_start(btile[:], bcat)
        eps_t = const.tile([128, 1], F32)
        nc.vector.memset(eps_t[:], EPS)
        ident = const.tile([128, 128], BF)
        make_identity(nc, ident[:])
        sw_t = const.tile([128, 1], F32)   # sum(W3) broadcast over partitions
        nc.gpsimd.dma_start(sw_t[:], scal[0:1, 0:1].to_broadcast((128, 1)))
        b3_t = const.tile([128, 1], F32)   # b3 broadcast
        nc.gpsimd.dma_start(b3_t[:], scal[0:1, 1:2].to_broadcast((128, 1)))
        # per-chunk stat rows: [p, {W3.qr, sum qr, sum qr^2}, chunk, j]
        Fp = const.tile([128, 3, nch, 4], F32)

        def W(col, ks, mc=0, width=128):
            return wtile[:, ks, col + mc * 128: col + mc * 128 + width]

        for c in range(nch):
            # ---- Stage A: load (f32->bf16 cast in DMA) + LayerNorm ----
            sT = []
            mv4 = apool.tile([128, 4, 2], F32, tag="mv")
            for bt in range(4):
                st = spool.tile([128, A * S], BF, tag="s_in")
                nc.gpsimd.dma_start(
                    st[:], s_in[c * NB + bt * 128: c * NB + (bt + 1) * 128, :])
                stats = apool.tile([128, 4, 6], F32, tag="stats")
                for g in range(4):
                    nc.vector.bn_stats(stats[:, g, :], st[:, g * 512:(g + 1) * 512])
                nc.vector.bn_aggr(mv4[:, bt], stats[:])
                sT.append(st)
            rt4 = apool.tile([128, 4], F32, tag="rt")
            nc.scalar.activation(rt4[:], mv4[:, :, 1], AF.Ln, bias=eps_t[:])
            nc.scalar.activation(rt4[:], rt4[:], AF.Exp, scale=-0.5)
            sN = []
            for bt in range(4):
                sn = spool.tile([128, A * S], BF, tag="sn")
                nc.vector.tensor_scalar(
                    sn[:], sT[bt][:], scalar1=mv4[:, bt, 0:1],
                    scalar2=rt4[:, bt:bt + 1],
                    op0=OP.subtract, op1=OP.mult)
                sN.append(sn)

            # ---- Stage T: PE transpose to feature-major ----
            # snT[p, fb, bt, bb] = sn_bt[bb, fb*128+p]; feature f = fb*128+p,
            # fb = 2*a + ks (a=agent, ks=K-half); sample index = bt*128+bb
            snT = tpool.tile([128, 16, 4, 128], BF, tag="snT")
            for fb in range(16):
                pt = psT.tile([128, 4, 128], BF, tag="ptrans")
                for bt in range(4):
                    nc.tensor.transpose(
                        pt[:, bt, :], sN[bt][:, fb * 128:(fb + 1) * 128], ident[:])
                if fb % 2 == 0:
                    nc.scalar.activation(snT[:, fb], pt[:], AF.Copy)
                else:
                    nc.vector.tensor_copy(snT[:, fb], pt[:])

            def rhs_s(ks, a):
                return snT[:, 2 * a + ks]   # [128, 4, 128] -> N=512

            # ---- projections (all contract over the 256 encoder inputs) ----
            def proj256(colbase, a, bias_slot, dst_mc_ap, func=AF.Identity):
                for mc in range(2):
                    ps = psA.tile([128, 2, NB], F32, tag="psmm")
                    for ks in range(2):
                        nc.tensor.matmul(
                            ps[:, 0], W(colbase, ks, mc), rhs_s(ks, a),
                            start=(ks == 0), stop=(ks == 1))
                    nc.scalar.activation(
                        dst_mc_ap(mc), ps[:, 0], func,
                        bias=btile[:, mc, bias_slot:bias_slot + 1])

            s_iT = mmout.tile([128, 2, NB], BF, tag="s_iT")
            proj256(C_WE, 0, B_BE, lambda mc: s_iT[:, mc])
            qT = mmout.tile([128, 2, NB], BF, tag="qT")
            proj256(C_WQ, 0, B_BQ, lambda mc: qT[:, mc])

            kT = kvpool.tile([128, 2, 7, NB], BF, tag="kT")
            vT = kvpool.tile([128, 2, 7, NB], BF, tag="vT")
            # a-pairs share one 2-bank PSUM tile and a single wide eviction
            apairs = [(1, 2), (3, 4), (5, 6), (7,)]
            for ap_ in apairs:
                for mc in range(2):
                    na = len(ap_)
                    psk = psA.tile([128, 2, NB], F32, tag="psmm")
                    for j, a in enumerate(ap_):
                        for ks in range(2):
                            nc.tensor.matmul(
                                psk[:, j], W(C_WK, ks, mc), rhs_s(ks, a),
                                start=(ks == 0), stop=(ks == 1))
                    nc.scalar.activation(
                        kT[:, mc, ap_[0] - 1:ap_[0] - 1 + na], psk[:, :na],
                        AF.Identity, bias=btile[:, mc, B_BK:B_BK + 1])
                    psv = psA.tile([128, 2, NB], F32, tag="psmm")
                    for j, a in enumerate(ap_):
                        for ks in range(2):
                            nc.tensor.matmul(
                                psv[:, j], W(C_WV, ks, mc), rhs_s(ks, a),
                                start=(ks == 0), stop=(ks == 1))
                    if USE_LRELU:
                        nc.scalar.activation(
                            vT[:, mc, ap_[0] - 1:ap_[0] - 1 + na], psv[:, :na],
                            AF.Lrelu, bias=btile[:, mc, B_BV:B_BV + 1],
                            alpha=0.01)
                    else:
                        # leaky_relu(t, .01) = max(.01*t, t), t = x + bv
                        dst = vT[:, mc, ap_[0] - 1:ap_[0] - 1 + na]
                        nc.scalar.activation(
                            dst, psv[:, :na], AF.Identity,
                            bias=btile[:, mc, B_BV:B_BV + 1])
                        nc.vector.scalar_tensor_tensor(
                            dst, dst, 0.01, dst, op0=OP.mult, op1=OP.max)

            # ---- attention ----
            qk = qkpool.tile([128, 2, 7, NB], BF, tag="qku")
            for k in range(7):
                nc.vector.tensor_mul(qk[:, :, k], kT[:, :, k], qT[:])
            # block-ones matmul: reduces QK over each head's 64 dims and
            # broadcasts the score back across them (diag chunks only)
            eb = kvpool.tile([128, 2, 7, NB], BF, tag="eb")
            kpairs = [(0, 1), (2, 3), (4, 5), (6,)]
            for mc in range(2):
                for kp in kpairs:
                    nk = len(kp)
                    pss = psA.tile([128, 2, NB], F32, tag="psmm")
                    for j, k in enumerate(kp):
                        nc.tensor.matmul(
                            pss[:, j], W(C_L, mc, mc), qk[:, mc, k],
                            start=True, stop=True)
                    nc.scalar.activation(
                        eb[:, mc, kp[0]:kp[0] + nk], pss[:, :nk], AF.Exp,
                        scale=1.0 / np.sqrt(HD))

            u = qkpool.tile([128, 2, 7, NB], BF, tag="qku")
            for k in range(7):
                nc.vector.tensor_mul(u[:, :, k], eb[:, :, k], vT[:, :, k])

            def ktree(src, dst):
                t0 = trpool.tile([128, 2, NB], BF, tag="tr0")
                nc.vector.tensor_add(t0[:], src[:, :, 0], src[:, :, 1])
                t1 = trpool.tile([128, 2, NB], BF, tag="tr1")
                nc.vector.tensor_add(t1[:], src[:, :, 2], src[:, :, 3])
                t2 = trpool.tile([128, 2, NB], BF, tag="tr2")
                nc.vector.tensor_add(t2[:], src[:, :, 4], src[:, :, 5])
                nc.vector.tensor_add(t0[:], t0[:], t1[:])
                nc.vector.tensor_add(t2[:], t2[:], src[:, :, 6])
                nc.vector.tensor_add(dst[:], t0[:], t2[:])

            avU = mmout.tile([128, 2, NB], BF, tag="avU")
            ktree(u, avU)
            sumB = mmout.tile([128, 2, NB], BF, tag="sumB")
            ktree(eb, sumB)
            rs = mmout.tile([128, 2, NB], BF, tag="rs")
            nc.scalar.activation(rs[:], sumB[:], AF.Ln)
            nc.scalar.activation(rs[:], rs[:], AF.Exp, scale=-1.0)
            avT = mmout.tile([128, 2, NB], BF, tag="avT")
            nc.vector.tensor_mul(avT[:], avU[:], rs[:])

            # ---- l1 fused with fc_out: qr = relu(W1a.s_i + WoW1b.av + b1f) ----
            qr = mmout.tile([128, 2, NB], BF, tag="qr")
            for mc in range(2):
                ps = psA.tile([128, 2, NB], F32, tag="psmm")
                for ks in range(2):
                    nc.tensor.matmul(ps[:, 0], W(C_W1A, ks, mc), s_iT[:, ks],
                                     start=(ks == 0), stop=False)
                for ks in range(2):
                    nc.tensor.matmul(ps[:, 0], W(C_WO1B, ks, mc), avT[:, ks],
                                     start=False, stop=(ks == 1))
                nc.scalar.activation(qr[:, mc], ps[:, 0], AF.Relu,
                                     bias=btile[:, mc, B_B1F:B_B1F + 1])
            qr2 = mmout.tile([128, 2, NB], BF, tag="qr2")
            nc.scalar.activation(qr2[:], qr[:], AF.Square)

            # ---- final LN+l3 stats via float32r matvecs ----
            ps1 = psF.tile([2, NB], F32, tag="ps1")
            for ks in range(2):
                nc.tensor.matmul(
                    ps1[:], W(C_W3O, ks, 0, width=2), qr[:, ks],
                    start=(ks == 0), stop=(ks == 1))
            ps2 = psF.tile([1, NB], F32, tag="ps2")
            for ks in range(2):
                nc.tensor.matmul(
                    ps2[:], W(C_W3O + 1, ks, 0, width=1), qr2[:, ks],
                    start=(ks == 0), stop=(ks == 1))
            stmp1 = fpool.tile([2, NB], F32, tag="stmp1")
            nc.scalar.activation(stmp1[:], ps1[:], AF.Copy)
            stmp2 = fpool.tile([1, NB], F32, tag="stmp2")
            nc.scalar.activation(stmp2[:], ps2[:], AF.Copy)
            # scatter row [1, 512] -> Fp[:, r, c, :] (sample = p*4 + j; the DMA
            # pairs the flat source stream with the partition-major dest)
            nc.gpsimd.dma_start(Fp[:, 0, c, :], stmp1[0:1, :])
            nc.gpsimd.dma_start(Fp[:, 1, c, :], stmp1[1:2, :])
            nc.gpsimd.dma_start(Fp[:, 2, c, :], stmp2[0:1, :])

        # ---- final LN+l3 math on [128, nch*4] ----
        FW = nch * 4
        w3qr = Fp[:, 0].rearrange("p c j -> p (c j)")
        sq = Fp[:, 1].rearrange("p c j -> p (c j)")
        sq2 = Fp[:, 2].rearrange("p c j -> p (c j)")
        m = fpool.tile([128, FW], F32, tag="fm")
        nc.scalar.mul(m[:], sq, 1.0 / H)
        ex2 = fpool.tile([128, FW], F32, tag="fe")
        nc.scalar.mul(ex2[:], sq2, 1.0 / H)
        var = fpool.tile([128, FW], F32, tag="fv")
        nc.vector.tensor_mul(var[:], m[:], m[:])
        nc.vector.tensor_sub(var[:], ex2[:], var[:])
        rstd = fpool.tile([128, FW], F32, tag="fr")
        nc.scalar.activation(rstd[:], var[:], AF.Ln, bias=eps_t[:])
        nc.scalar.activation(rstd[:], rstd[:], AF.Exp, scale=-0.5)
        msw = fpool.tile([128, FW], F32, tag="fw")
        nc.vector.tensor_scalar_mul(msw[:], m[:], sw_t[:])
        res = fpool.tile([128, FW], F32, tag="fres")
        nc.vector.tensor_sub(res[:], w3qr, msw[:])
        nc.vector.tensor_mul(res[:], res[:], rstd[:])
        nc.vector.tensor_scalar_add(res[:], res[:], b3_t[:])
        nc.sync.dma_start(
            out.rearrange("(c p j) -> p c j", p=128, j=4),
            res.rearrange("p (c j) -> p c j", j=4))
    return nc


def _prepare_host(We, be, Wq, Wk, Wv, bv, Wo, bo, W1, b1, W3, b3):
    f = lambda x: np.asarray(x, dtype=np.float32)
    We, be, Wq, Wk, Wv, bv = f(We), f(be), f(Wq), f(Wk), f(Wv), f(bv)
    Wo, bo, W1, b1, W3, b3 = f(Wo), f(bo), f(W1), f(b1), f(W3), f(b3)
    WeQ, beQ = We @ Wq, be @ Wq
    WeK, beK = We @ Wk, be @ Wk
    WeV, beV = We @ Wv, be @ Wv + bv
    W1a, W1b = W1[:D], W1[D:]
    WoW1b, b1f = Wo @ W1b, b1 + bo @ W1b
    L = np.zeros((H, H), np.float32)
    for n in range(NH):
        L[n * HD:(n + 1) * HD, n * HD:(n + 1) * HD] = 1.0
    w3o = np.zeros((H, 2), np.float32)
    w3o[:, 0] = W3[:, 0]
    w3o[:, 1] = 1.0
    wfull = np.concatenate([We, WeQ, WeK, WeV, W1a, WoW1b, L, w3o], axis=1)
    assert wfull.shape == (256, NW)
    wcat = np.ascontiguousarray(
        wfull.reshape(2, 128, NW).transpose(1, 0, 2)).astype(ml_dtypes.bfloat16)
    ones = np.ones(H, np.float32)
    bfull = np.stack([be, beQ, beK, beV, b1f, W3[:, 0], ones], axis=1)
    assert bfull.shape == (256, NBI)
    bcat = np.ascontiguousarray(bfull.reshape(2, 128, NBI).transpose(1, 0, 2))
    scal = np.array([[W3.sum(), b3[0]]], np.float32)
    return wcat, bcat, scal


_CACHED = {}


def _get_compiled(nch=NCH, num_devices=1):
    key = (nch, num_devices)
    if key not in _CACHED:
        nc = bacc.Bacc("TRN2", target_bir_lowering=False, debug=False,
                       num_devices=num_devices)
        with tile.TileContext(nc) as tc:
            build_kernel(tc, nch=nch)
        nc.compile()
        _CACHED[key] = nc
    return _CACHED[key]


def kernel(s, We, be, Wq, Wk, Wv, bv, Wo, bo, W1, b1, W3, b3, _trace=False):
    s = np.asarray(s, dtype=np.float32)
    wcat, bcat, scal = _prepare_host(We, be, Wq, Wk, Wv, bv, Wo, bo, W1, b1,
                                     W3, b3)
    nc = _get_compiled()
    in_maps = []
    for i in range(NCORES):
        shard = np.ascontiguousarray(s[i * BC:(i + 1) * BC])
        in_maps.append({"s": shard, "wcat": wcat, "bcat": bcat, "scal": scal})
    res = run_bass_kernel_spmd(nc, in_maps, core_ids=list(range(NCORES)),
                               trace=_trace)
    outs = [np.asarray(r["out"], np.float32).reshape(BC, 1)
            for r in res.results]
    full = np.concatenate(outs, axis=0)
    if _trace:
        return full, res
    return full
